# revision 21
# baseline (speedup 1.0000x reference)
"""Trainium2 Bass kernel for CascadedQuantization (residual VQ forward).

Math: per cascade level l and channel-group g, each residual vector r (D=64)
is assigned its nearest codeword (argmin ||r-c||^2 over K=2048) and the
codeword is subtracted to form the next residual. Forward output is the sum
of assigned codewords = x - r_final (softmax STE terms cancel exactly in
forward). argmin d2 == argmax (r.c - 0.5*||c||^2), so each stage is one
[N,D]x[D,K] matmul with a fused norm row, an argmax over K, and a gather.

Sharding: data-parallel over batch B=8 -> 8 cores, no collectives.

Layout per core (B slice = 1):
  rT[g]  [65, 1024] SBUF: rows 0..63 = residual (transposed, matmul-ready),
         row 64 = constant 1.0 (multiplies the -0.5*||c||^2 row of cbta).
  cbta   [65, 2048] per (l,g), streamed from HBM: rows 0..63 = cb^T,
         row 64 = -0.5*||c_k||^2.
  scores [128, 2048] per pixel-chunk via 4 fp32 matmuls (free dim 512).
  argmax via DVE max (top-8) + max_index; slot-0 index per chunk lands in
  X3[:, 0, j] so X3[:, 0, :] is the [128, 8] u32 offset tile for a single
  SWDGE indirect-DMA gather of all 1024 codeword rows (256B each) from HBM.
  Gathered q [128, 8, 64] is PE-transposed back to [64, 1024] and subtracted
  from rT on GPSIMD. Final out = x - rT.
"""

import numpy as np

B, C, H, W = 8, 256, 32, 32
L, G, K, D = 4, 4, 2048, 64
N = H * W  # 1024
NCHUNKS = N // 128  # 8
KC = K // 512  # 4 matmul free-dim chunks

_CACHE = {}


def _register_argmax():
    """Register a custom single-pass DVE argmax: accum_out[p] = last index k
    with in0[p,k] == running max == global max (== np.argmax when the row has
    no exact fp32 ties, which holds for this input: verified 0/131072 rows).
    Halves the DVE cost vs the stock max + max_index two-pass pair."""
    import numpy as np
    from concourse import dve_ops
    from concourse.dve_spec import AluOp, Idx, MaxNeg, Spec, Src0, lower, scan, select
    from concourse.dve_uop import DveOpSpec

    name = "ARGMAX_LAST_ANT"
    for o in dve_ops.OPS:
        if o.name == name:
            return o

    m = scan(AluOp.MAX, Src0)
    body = select(Src0 >= m, Idx, MaxNeg)

    def ref(in0, in1, c0, c1, c2):
        rm = np.maximum.accumulate(in0, axis=-1)
        idxs = np.arange(in0.shape[-1], dtype=np.float32)
        out = np.where(in0 >= rm, idxs, -np.finfo(np.float32).max)
        return out, out.max(axis=-1)

    spec = Spec(body=body, accum=AluOp.MAX, reference=ref)
    opcode = dve_ops._CUSTOM_DVE_ROW_BASE + len(dve_ops.OPS)
    dve_ops._SUB_OPCODE_FOR_NAME[name] = opcode
    shas = {}
    for ver in ("v3", "v4"):
        u = lower(spec, ver=ver)
        shas[ver] = DveOpSpec(name=name, opcode=opcode, uops=u, rd1_en=False).sha(ver)
    op = dve_ops.DveOp(name, spec, subdim=False, uops_sha=shas)
    dve_ops.OPS.append(op)
    dve_ops.CUSTOM_DVE_SPECS[name] = spec
    return op


def _build(repeats=1):
    import concourse.bacc as bacc
    import concourse.bass as bass
    import concourse.mybir as mybir
    import concourse.tile as tile
    from concourse import library_config

    f32 = mybir.dt.float32
    u16 = mybir.dt.uint16
    i16 = mybir.dt.int16

    argmax_op = _register_argmax()

    # Bacc (not bass.Bass): its compile() runs move_matmul_waits_to_ldweights
    # + generate_event_semaphores, which walrus needs (<=2 waits per Matmult).
    nc = bacc.Bacc("TRN2", target_bir_lowering=False, debug=False)

    # x is pre-augmented on host: [G, 65, N], row 64 = 1.0 (the "ones" row
    # that multiplies the -0.5*||c||^2 row of cbta inside the matmul). Loading
    # it by DMA (instead of DMA + memset) keeps the first matmul at <=2 sync
    # waits (walrus Matmult limit).
    x_d = nc.dram_tensor("x", [G, 65, N], f32, kind="ExternalInput")
    cbta_d = nc.dram_tensor("cbta", [L * G, 65, K], f32, kind="ExternalInput")
    cbrows_d = nc.dram_tensor("cbrows", [L * G * K, D], f32, kind="ExternalInput")
    ident_d = nc.dram_tensor("ident", [128, 128], f32, kind="ExternalInput")
    out_d = nc.dram_tensor("out", [C, N], f32, kind="ExternalOutput")

    with tile.TileContext(nc) as tc:
        with (
            tc.tile_pool(name="persist", bufs=1) as persist,
            tc.tile_pool(name="cbta", bufs=3) as cbta_pool,
            tc.tile_pool(name="scores", bufs=3) as scores_pool,
            tc.tile_pool(name="small", bufs=2) as small_pool,
            tc.tile_pool(name="qpool", bufs=2) as q_pool,
            tc.tile_pool(name="io", bufs=2) as io_pool,
            tc.tile_pool(name="ps_s", bufs=2, space=bass.MemorySpace.PSUM) as ps_s,
            tc.tile_pool(name="ps_t", bufs=2, space=bass.MemorySpace.PSUM) as ps_t,
            tc.tile_pool(name="dram", bufs=2, space=bass.MemorySpace.DRAM) as dram_pool,
        ):
            # GPSIMD is used only for dma_gather (mlp library); load it once.
            nc.gpsimd.load_library(library_config.mlp)

            ident = persist.tile([128, 128], f32, tag="ident")
            nc.sync.dma_start(out=ident[:], in_=ident_d[:])

            rT = []
            for g in range(G):
                t = persist.tile([65, N], f32, tag=f"rT{g}")
                rT.append(t)

            def load_x():
                for g in range(G):
                    nc.sync.dma_start(out=rT[g][:], in_=x_d[g, :, :])

            load_x()

            def load_cbta(lg):
                cbta = cbta_pool.tile([65, K], f32, tag="cbta")
                nc.sync.dma_start(out=cbta[:], in_=cbta_d[lg, :, :])
                return cbta

            def finish_group(pend):
                """Transpose gathered codewords and update the residual for a
                previous (l, g) step. Emitted one step late so the PE (strict
                in-order FIFO) never stalls waiting for that step's gather."""
                g, q, qT = pend
                qtp = ps_t.tile([64, NCHUNKS, 128], f32, tag="qtp")
                for j in range(NCHUNKS):
                    nc.tensor.transpose(qtp[:, j, :], q[:, j, :], ident[:])
                    nc.scalar.copy(
                        out=qT[:, j * 128 : (j + 1) * 128], in_=qtp[:, j, :]
                    )
                # residual update (DVE: gpsimd's tensor_tensor lives in a
                # different ucode library than dma_gather)
                nc.vector.tensor_sub(rT[g][0:64, :], rT[g][0:64, :], qT[:])

            steps = [(l, g) for _ in range(repeats) for l in range(L) for g in range(G)]
            cbta_tiles = {0: load_cbta(0)}
            pending = None
            for si, (l, g) in enumerate(steps):
                    if si > 0 and l == 0 and g == 0:
                        # next benchmark repeat: reset residuals to x
                        if pending is not None:
                            finish_group(pending)
                            pending = None
                        load_x()
                    lg = l * G + g
                    cbta = cbta_tiles.pop(si)
                    # prefetch the next step's codebook ahead of this step's
                    # index-DMA chain (both go through the SP sequencer FIFO)
                    if si + 1 < len(steps):
                        cbta_tiles[si + 1] = load_cbta(steps[si + 1][0] * G + steps[si + 1][1])

                    Xf = small_pool.tile([128, NCHUNKS], f32, tag="Xf")
                    Xu = small_pool.tile([128, NCHUNKS], u16, tag="Xu")
                    q = q_pool.tile([128, NCHUNKS, D], f32, tag="q")
                    qT = q_pool.tile([64, N], f32, tag="qT")

                    for j in range(NCHUNKS):
                        lhsT = rT[g][:, j * 128 : (j + 1) * 128]
                        scores = scores_pool.tile([128, K], f32, tag="scores")
                        amout = scores_pool.tile([128, K], f32, tag="amout")
                        for h in range(2):
                            ps = ps_s.tile([128, 2, 512], f32, tag="ps_s")
                            for c in range(2):
                                kc = h * 2 + c
                                nc.tensor.matmul(
                                    ps[:, c, :],
                                    lhsT,
                                    cbta[:, kc * 512 : (kc + 1) * 512],
                                    start=True,
                                    stop=True,
                                )
                            nc.scalar.copy(
                                out=scores[:, h * 1024 : (h + 1) * 1024],
                                in_=ps[:, :, :],
                            )
                        # single-pass argmax -> float index in Xf[:, j]
                        nc.vector._custom_dve(
                            argmax_op,
                            out=amout[:],
                            in0=scores[:],
                            accum_out=Xf[:, j : j + 1],
                        )
                    # cast float indices to u16 for the gather index tile
                    nc.scalar.copy(out=Xu[:], in_=Xf[:])

                    # previous step's epilogue goes to the engine FIFOs here,
                    # after this step's matmuls/argmaxes
                    if pending is not None:
                        finish_group(pending)

                    # Build the dma_gather index tile: list position i = pixel
                    # n lives at T[n % 16, n // 16] (int16, replicated to all
                    # 128 partitions). The 128-partition -> 16-partition fold
                    # goes through a DRAM round trip with a strided re-read:
                    # T[q, j*8+a] = Xu[16a+q, j].
                    idxd = dram_pool.tile([128, NCHUNKS], u16, tag="idxd")
                    nc.sync.dma_start(out=idxd[:], in_=Xu[:])
                    T = small_pool.tile([128, 64], u16, tag="T")
                    nc.sync.dma_start(
                        out=T[0:16, :].rearrange("q (j a) -> q j a", j=NCHUNKS),
                        in_=idxd[:].rearrange("(a q) j -> q j a", a=8),
                    )
                    nc.sync.dma_start(out=T[16:32, :], in_=T[0:16, :])
                    nc.sync.dma_start(out=T[32:64, :], in_=T[0:32, :])
                    nc.sync.dma_start(out=T[64:128, :], in_=T[0:64, :])

                    # gather all 1024 nearest codewords for this (l, g)
                    nc.gpsimd.dma_gather(
                        out_ap=q[:],
                        in_ap=cbrows_d[lg * K : (lg + 1) * K, :],
                        idxs_ap=T[:].bitcast(i16),
                        num_idxs=N,
                        num_idxs_reg=N,
                        elem_size=D,
                    )
                    pending = (g, q, qT)

            finish_group(pending)

            # out = x - r_final  (xhat accumulated implicitly)
            for g in range(G):
                xin = io_pool.tile([64, N], f32, tag="xin")
                nc.sync.dma_start(out=xin[:], in_=x_d[g, 0:64, :])
                xout = io_pool.tile([64, N], f32, tag="xout")
                nc.vector.tensor_sub(xout[:], xin[:], rT[g][0:64, :])
                nc.sync.dma_start(out=out_d[g * 64 : (g + 1) * 64, :], in_=xout[:])

    nc.compile()
    return nc


def _prep_inputs(x, codebooks):
    x = np.ascontiguousarray(np.asarray(x, dtype=np.float32))
    cb = np.ascontiguousarray(np.asarray(codebooks, dtype=np.float32))
    cbta = np.empty((L * G, 65, K), dtype=np.float32)
    for l in range(L):
        for g in range(G):
            cbta[l * G + g, 0:64, :] = cb[l, g].T
            cbta[l * G + g, 64, :] = -0.5 * np.sum(
                cb[l, g].astype(np.float32) ** 2, axis=1, dtype=np.float32
            )
    cbrows = np.ascontiguousarray(cb.reshape(L * G * K, D))
    ident = np.eye(128, dtype=np.float32)
    in_maps = []
    for b in range(B):
        xa = np.ones((G, 65, N), dtype=np.float32)
        xa[:, 0:64, :] = x[b].reshape(G, 64, N)
        in_maps.append(
            {
                "x": xa,
                "cbta": cbta,
                "cbrows": cbrows,
                "ident": ident,
            }
        )
    return in_maps


def kernel(x, codebooks, _trace=False, _trace_kwargs=None):
    from concourse.bass_utils import run_bass_kernel_spmd

    if "nc" not in _CACHE:
        _CACHE["nc"] = _build()
    nc = _CACHE["nc"]

    in_maps = _prep_inputs(x, codebooks)
    res = run_bass_kernel_spmd(
        nc,
        in_maps,
        core_ids=list(range(B)),
        trace=_trace,
        **(_trace_kwargs or {}),
    )
    _CACHE["last_result"] = res
    out = np.stack([np.asarray(res.results[b]["out"]) for b in range(B)], axis=0)
    return out.reshape(B, C, H, W).astype(np.float32)


# revision 22
# speedup vs baseline: 1.1090x; 1.1090x over previous
"""Trainium2 Bass kernel for CascadedQuantization (residual VQ forward).

Math: per cascade level l and channel-group g, each residual vector r (D=64)
is assigned its nearest codeword (argmin ||r-c||^2 over K=2048) and the
codeword is subtracted to form the next residual. Forward output is the sum
of assigned codewords = x - r_final (softmax STE terms cancel exactly in
forward). argmin d2 == argmax (r.c - 0.5*||c||^2).

Sharding: data-parallel over batch B=8 -> 8 cores, no collectives.

Per-core pipeline, per (level, group):
  - scores r.c via fp32 matmuls with C=64 contraction, row-packed in PAIRS
    (tile_position row-groups 0-63 / 64-127 run concurrently on the PE
    array; measured ~388 ns per pair of [64,128]x[64,512] fp32 matmuls).
    Residuals rT and codebooks cbta2 are stored DUPLICATED in both
    partition halves to feed the two row groups.
  - custom single-pass DVE op: argmax_k(Src0 + Src1) reading scores
    directly from PSUM (Src0) and the broadcast -0.5*||c||^2 row (Src1),
    returning the (last-tie) argmax index as fp32 in accum_out. Exact-tie
    rows verified absent for this input (0/131072).
  - one dma_gather fetches all 1024 selected codewords (256B rows) from
    HBM; PE transposes them back to [D, N]; DVE subtracts; a small DMA
    duplicates the updated residual into partitions 64-127.
  - the gather index tile needs a 128->16 partition fold (wrap-by-16
    int16 layout): done via a DRAM round-trip with a strided re-read.
  - per-group epilogue is emitted one step late so the strictly in-order
    PE FIFO never waits on the gather chain.
Output = x - r_final.
"""

import numpy as np

B, C, H, W = 8, 256, 32, 32
L, G, K, D = 4, 4, 2048, 64
N = H * W  # 1024
NCHUNKS = N // 128  # 8
KC = K // 512  # 4 matmul free-dim chunks

_CACHE = {}


def _register_argmax_bias():
    """Custom single-pass DVE argmax-with-bias: accum_out[p] = last index k
    attaining max_k (in0[p,k] + in1[p,k]), as float. First/last tie choice
    is irrelevant for this input (no exact fp32 ties)."""
    import numpy as np
    from concourse import dve_ops
    from concourse.dve_spec import (
        AluOp, Idx, MaxNeg, Spec, Src0, Src1, lower, scan, select,
    )
    from concourse.dve_uop import DveOpSpec

    name = "ARGMAX_BIAS_ANT"
    for o in dve_ops.OPS:
        if o.name == name:
            return o

    s = Src0 + Src1
    body = select(s >= scan(AluOp.MAX, s), Idx, MaxNeg)

    def ref(in0, in1, c0, c1, c2):
        t = in0 + in1
        rm = np.maximum.accumulate(t, axis=-1)
        idxs = np.arange(t.shape[-1], dtype=np.float32)
        out = np.where(t >= rm, idxs, -np.finfo(np.float32).max)
        return out, out.max(axis=-1)

    spec = Spec(body=body, accum=AluOp.MAX, reference=ref)
    opcode = dve_ops._CUSTOM_DVE_ROW_BASE + len(dve_ops.OPS)
    dve_ops._SUB_OPCODE_FOR_NAME[name] = opcode
    shas = {}
    for ver in ("v3", "v4"):
        u = lower(spec, ver=ver)
        shas[ver] = DveOpSpec(name=name, opcode=opcode, uops=u, rd1_en=True).sha(ver)
    op = dve_ops.DveOp(name, spec, subdim=False, uops_sha=shas)
    dve_ops.OPS.append(op)
    dve_ops.CUSTOM_DVE_SPECS[name] = spec
    return op


def _build(repeats=1):
    import concourse.bacc as bacc
    import concourse.bass as bass
    import concourse.mybir as mybir
    import concourse.tile as tile
    from concourse import library_config

    f32 = mybir.dt.float32
    u16 = mybir.dt.uint16
    i16 = mybir.dt.int16

    argmax_op = _register_argmax_bias()

    # Bacc (not bass.Bass): its compile() runs move_matmul_waits_to_ldweights
    # + generate_event_semaphores, which walrus needs (<=2 waits per Matmult).
    nc = bacc.Bacc("TRN2", target_bir_lowering=False, debug=False)

    x_d = nc.dram_tensor("x", [G, 64, N], f32, kind="ExternalInput")
    # cbta2[lg]: rows 0..63 = cb[l,g].T, rows 64..127 = the same (row-group 2)
    cbta2_d = nc.dram_tensor("cbta2", [L * G, 128, K], f32, kind="ExternalInput")
    norms_d = nc.dram_tensor("norms", [L * G, K], f32, kind="ExternalInput")
    cbrows_d = nc.dram_tensor("cbrows", [L * G * K, D], f32, kind="ExternalInput")
    ident_d = nc.dram_tensor("ident", [128, 128], f32, kind="ExternalInput")
    out_d = nc.dram_tensor("out", [C, N], f32, kind="ExternalOutput")

    with tile.TileContext(nc) as tc:
        with (
            tc.tile_pool(name="persist", bufs=1) as persist,
            tc.tile_pool(name="cbta", bufs=3) as cbta_pool,
            tc.tile_pool(name="nrm", bufs=3) as nrm_pool,
            tc.tile_pool(name="amout", bufs=1) as am_pool,
            tc.tile_pool(name="small", bufs=2) as small_pool,
            tc.tile_pool(name="qpool", bufs=2) as q_pool,
            tc.tile_pool(name="io", bufs=2) as io_pool,
            tc.tile_pool(name="ps_s", bufs=2, space=bass.MemorySpace.PSUM) as ps_s,
            tc.tile_pool(name="dram", bufs=2, space=bass.MemorySpace.DRAM) as dram_pool,
        ):
            # GPSIMD is used only for dma_gather (mlp library); load it once.
            nc.gpsimd.load_library(library_config.mlp)

            ident = persist.tile([128, 128], f32, tag="ident")
            nc.sync.dma_start(out=ident[:], in_=ident_d[:])

            rT = []
            for g in range(G):
                t = persist.tile([128, N], f32, tag=f"rT{g}")
                rT.append(t)

            def load_x():
                for g in range(G):
                    nc.sync.dma_start(out=rT[g][0:64, :], in_=x_d[g, :, :])
                    nc.sync.dma_start(out=rT[g][64:128, :], in_=x_d[g, :, :])

            load_x()

            def load_cbta(lg):
                cbta = cbta_pool.tile([128, K], f32, tag="cbta")
                nc.sync.dma_start(out=cbta[:], in_=cbta2_d[lg, :, :])
                nrm = nrm_pool.tile([128, K], f32, tag="nrm")
                src = bass.AP(
                    norms_d[:].tensor, lg * K, [[0, 128], [1, K]]
                )  # partition-stride-0 broadcast of the norms row
                nc.sync.dma_start(out=nrm[:], in_=src)
                return cbta, nrm

            amout = am_pool.tile([128, K], f32, tag="amout")

            def finish_group(pend):
                """Transpose gathered codewords and update the residual for a
                previous (l, g) step. Emitted one step late so the PE (strict
                in-order FIFO) never stalls waiting for that step's gather."""
                g, q, qT = pend
                qtp = ps_s.tile([64, NCHUNKS, 128], f32, tag="ps_s")
                for j in range(NCHUNKS):
                    nc.tensor.transpose(qtp[:, j, :], q[:, j, :], ident[:])
                    nc.scalar.copy(
                        out=qT[:, j * 128 : (j + 1) * 128], in_=qtp[:, j, :]
                    )
                # residual update (DVE; gpsimd's tensor_tensor lives in a
                # different ucode library than dma_gather), then duplicate
                # the updated half into partitions 64-127 for row-group 2
                nc.vector.tensor_sub(rT[g][0:64, :], rT[g][0:64, :], qT[:])
                nc.sync.dma_start(out=rT[g][64:128, :], in_=rT[g][0:64, :])

            steps = [(l, g) for _ in range(repeats) for l in range(L) for g in range(G)]
            cbta_tiles = {0: load_cbta(steps[0][0] * G + steps[0][1])}
            pending = None
            for si, (l, g) in enumerate(steps):
                if si > 0 and l == 0 and g == 0:
                    # next benchmark repeat: reset residuals to x
                    if pending is not None:
                        finish_group(pending)
                        pending = None
                    load_x()
                lg = l * G + g
                cbta, nrm = cbta_tiles.pop(si)
                # prefetch the next step's codebook ahead of this step's
                # index-DMA chain (both go through the SP sequencer FIFO)
                if si + 1 < len(steps):
                    nl, ng = steps[si + 1]
                    cbta_tiles[si + 1] = load_cbta(nl * G + ng)

                Xf = small_pool.tile([128, NCHUNKS], f32, tag="Xf")
                Xu = small_pool.tile([128, NCHUNKS], u16, tag="Xu")
                q = q_pool.tile([128, NCHUNKS, D], f32, tag="q")
                qT = q_pool.tile([64, N], f32, tag="qT")

                for pj in range(NCHUNKS // 2):
                    j0, j1 = 2 * pj, 2 * pj + 1
                    psA = ps_s.tile([128, KC, 512], f32, tag="ps_s")
                    psB = ps_s.tile([128, KC, 512], f32, tag="ps_s")
                    for kc in range(KC):
                        # row-packed fp32 pair: row-groups 0-63 and 64-127
                        # compute chunks j0 and j1 concurrently
                        nc.tensor.matmul(
                            psA[:, kc, :],
                            rT[g][0:64, j0 * 128 : (j0 + 1) * 128],
                            cbta[0:64, kc * 512 : (kc + 1) * 512],
                            start=True,
                            stop=True,
                        )
                        nc.tensor.matmul(
                            psB[:, kc, :],
                            rT[g][64:128, j1 * 128 : (j1 + 1) * 128],
                            cbta[64:128, kc * 512 : (kc + 1) * 512],
                            start=True,
                            stop=True,
                        )
                    # single-pass biased argmax straight from PSUM
                    nc.vector._custom_dve(
                        argmax_op,
                        out=amout[:],
                        in0=psA[:, :, :],
                        in1=nrm[:],
                        accum_out=Xf[:, j0 : j0 + 1],
                    )
                    nc.vector._custom_dve(
                        argmax_op,
                        out=amout[:],
                        in0=psB[:, :, :],
                        in1=nrm[:],
                        accum_out=Xf[:, j1 : j1 + 1],
                    )
                # cast float indices to u16 for the gather index tile
                nc.scalar.copy(out=Xu[:], in_=Xf[:])

                # previous step's epilogue goes to the engine FIFOs here,
                # after this step's matmuls/argmaxes
                if pending is not None:
                    finish_group(pending)

                # Build the dma_gather index tile: list position i = pixel
                # n lives at T[n % 16, n // 16] (int16, replicated to all
                # 128 partitions). The 128-partition -> 16-partition fold
                # goes through a DRAM round trip with a strided re-read:
                # T[qq, j*8+a] = Xu[16a+qq, j].
                idxd = dram_pool.tile([128, NCHUNKS], u16, tag="idxd")
                nc.sync.dma_start(out=idxd[:], in_=Xu[:])
                T = small_pool.tile([128, 64], u16, tag="T")
                nc.sync.dma_start(
                    out=T[0:16, :].rearrange("q (j a) -> q j a", j=NCHUNKS),
                    in_=idxd[:].rearrange("(a q) j -> q j a", a=8),
                )
                nc.sync.dma_start(out=T[16:32, :], in_=T[0:16, :])
                nc.sync.dma_start(out=T[32:64, :], in_=T[0:32, :])
                nc.sync.dma_start(out=T[64:128, :], in_=T[0:64, :])

                # gather all 1024 nearest codewords for this (l, g)
                nc.gpsimd.dma_gather(
                    out_ap=q[:],
                    in_ap=cbrows_d[lg * K : (lg + 1) * K, :],
                    idxs_ap=T[:].bitcast(i16),
                    num_idxs=N,
                    num_idxs_reg=N,
                    elem_size=D,
                )
                pending = (g, q, qT)

            finish_group(pending)

            # out = x - r_final  (xhat accumulated implicitly)
            for g in range(G):
                xin = io_pool.tile([64, N], f32, tag="xin")
                nc.sync.dma_start(out=xin[:], in_=x_d[g, :, :])
                xout = io_pool.tile([64, N], f32, tag="xout")
                nc.vector.tensor_sub(xout[:], xin[:], rT[g][0:64, :])
                nc.sync.dma_start(out=out_d[g * 64 : (g + 1) * 64, :], in_=xout[:])

    nc.compile()
    return nc


def _prep_inputs(x, codebooks):
    x = np.ascontiguousarray(np.asarray(x, dtype=np.float32))
    cb = np.ascontiguousarray(np.asarray(codebooks, dtype=np.float32))
    cbta2 = np.empty((L * G, 128, K), dtype=np.float32)
    norms = np.empty((L * G, K), dtype=np.float32)
    for l in range(L):
        for g in range(G):
            ct = cb[l, g].T
            cbta2[l * G + g, 0:64, :] = ct
            cbta2[l * G + g, 64:128, :] = ct
            norms[l * G + g] = -0.5 * np.sum(
                cb[l, g].astype(np.float32) ** 2, axis=1, dtype=np.float32
            )
    cbrows = np.ascontiguousarray(cb.reshape(L * G * K, D))
    ident = np.eye(128, dtype=np.float32)
    in_maps = []
    for b in range(B):
        in_maps.append(
            {
                "x": np.ascontiguousarray(x[b].reshape(G, 64, N)),
                "cbta2": cbta2,
                "norms": norms,
                "cbrows": cbrows,
                "ident": ident,
            }
        )
    return in_maps


def kernel(x, codebooks, _trace=False, _trace_kwargs=None):
    from concourse.bass_utils import run_bass_kernel_spmd

    if "nc" not in _CACHE:
        _CACHE["nc"] = _build()
    nc = _CACHE["nc"]

    in_maps = _prep_inputs(x, codebooks)
    res = run_bass_kernel_spmd(
        nc,
        in_maps,
        core_ids=list(range(B)),
        trace=_trace,
        **(_trace_kwargs or {}),
    )
    _CACHE["last_result"] = res
    out = np.stack([np.asarray(res.results[b]["out"]) for b in range(B)], axis=0)
    return out.reshape(B, C, H, W).astype(np.float32)


# revision 25
# speedup vs baseline: 2.8017x; 2.5263x over previous
"""Trainium2 Bass kernel for CascadedQuantization (residual VQ forward).

Math: per cascade level l and channel-group g, each residual vector r (D=64)
is assigned its nearest codeword (argmin ||r-c||^2 over K=2048) and the
codeword is subtracted to form the next residual. Forward output is the sum
of assigned codewords = x - r_final (softmax STE terms cancel exactly in
forward). argmin d2 == argmax (r.c - 0.5*||c||^2).

Sharding: data-parallel over batch B=8 -> 8 cores, no collectives.

Per-core pipeline, per (level, group):
  - scores r.c via fp32 matmuls with C=64 contraction, row-packed in PAIRS
    (tile_position row-groups 0-63 / 64-127 run concurrently on the PE
    array; measured ~388 ns per pair of [64,128]x[64,512] fp32 matmuls).
    Residuals rT and codebooks cbta2 are stored DUPLICATED in both
    partition halves to feed the two row groups.
  - custom single-pass DVE op: argmax_k(Src0 + Src1) reading scores
    directly from PSUM (Src0) and the broadcast -0.5*||c||^2 row (Src1),
    returning the (last-tie) argmax index as fp32 in accum_out. Exact-tie
    rows verified absent for this input (0/131072).
  - one dma_gather fetches all 1024 selected codewords (256B rows) from
    HBM; PE transposes them back to [D, N]; DVE subtracts; a small DMA
    duplicates the updated residual into partitions 64-127.
  - the gather index tile needs a 128->16 partition fold (wrap-by-16
    int16 layout): done via a DRAM round-trip with a strided re-read.
  - per-group epilogue is emitted one step late so the strictly in-order
    PE FIFO never waits on the gather chain.
Output = x - r_final.
"""

import numpy as np

B, C, H, W = 8, 256, 32, 32
L, G, K, D = 4, 4, 2048, 64
N = H * W  # 1024
NCHUNKS = N // 128  # 8
KC = K // 512  # 4 matmul free-dim chunks

_CACHE = {}


def _register_argmax_bias():
    """Custom single-pass DVE argmax-with-bias: accum_out[p] = last index k
    attaining max_k (in0[p,k] + in1[p,k]), as float. First/last tie choice
    is irrelevant for this input (no exact fp32 ties)."""
    import numpy as np
    from concourse import dve_ops
    from concourse.dve_spec import (
        AluOp, Idx, MaxNeg, Spec, Src0, Src1, lower, scan, select,
    )
    from concourse.dve_uop import DveOpSpec

    name = "ARGMAX_BIAS_ANT"
    for o in dve_ops.OPS:
        if o.name == name:
            return o

    s = Src0 + Src1
    body = select(s >= scan(AluOp.MAX, s), Idx, MaxNeg)

    def ref(in0, in1, c0, c1, c2):
        t = in0 + in1
        rm = np.maximum.accumulate(t, axis=-1)
        idxs = np.arange(t.shape[-1], dtype=np.float32)
        out = np.where(t >= rm, idxs, -np.finfo(np.float32).max)
        return out, out.max(axis=-1)

    spec = Spec(body=body, accum=AluOp.MAX, reference=ref)
    opcode = dve_ops._CUSTOM_DVE_ROW_BASE + len(dve_ops.OPS)
    dve_ops._SUB_OPCODE_FOR_NAME[name] = opcode
    shas = {}
    for ver in ("v3", "v4"):
        u = lower(spec, ver=ver)
        shas[ver] = DveOpSpec(name=name, opcode=opcode, uops=u, rd1_en=True).sha(ver)
    op = dve_ops.DveOp(name, spec, subdim=False, uops_sha=shas)
    dve_ops.OPS.append(op)
    dve_ops.CUSTOM_DVE_SPECS[name] = spec
    return op


def _build(repeats=1):
    import concourse.bacc as bacc
    import concourse.bass as bass
    import concourse.mybir as mybir
    import concourse.tile as tile
    from concourse import library_config

    f32 = mybir.dt.float32
    u16 = mybir.dt.uint16
    i16 = mybir.dt.int16

    argmax_op = _register_argmax_bias()

    # Bacc (not bass.Bass): its compile() runs move_matmul_waits_to_ldweights
    # + generate_event_semaphores, which walrus needs (<=2 waits per Matmult).
    nc = bacc.Bacc("TRN2", target_bir_lowering=False, debug=False)

    x_d = nc.dram_tensor("x", [G, 64, N], f32, kind="ExternalInput")
    # cbta2[lg]: rows 0..63 = cb[l,g].T, rows 64..127 = the same (row-group 2)
    cbta2_d = nc.dram_tensor("cbta2", [L * G, 128, K], f32, kind="ExternalInput")
    norms_d = nc.dram_tensor("norms", [L * G, K], f32, kind="ExternalInput")
    cbrows_d = nc.dram_tensor("cbrows", [L * G * K, D], f32, kind="ExternalInput")
    ident_d = nc.dram_tensor("ident", [128, 128], f32, kind="ExternalInput")
    out_d = nc.dram_tensor("out", [C, N], f32, kind="ExternalOutput")

    with tile.TileContext(nc) as tc:
        with (
            tc.tile_pool(name="persist", bufs=1) as persist,
            tc.tile_pool(name="cbta", bufs=3) as cbta_pool,
            tc.tile_pool(name="nrm", bufs=3) as nrm_pool,
            tc.tile_pool(name="amout", bufs=1) as am_pool,
            tc.tile_pool(name="scores", bufs=4) as scores_pool,
            tc.tile_pool(name="small", bufs=2) as small_pool,
            tc.tile_pool(name="qpool", bufs=2) as q_pool,
            tc.tile_pool(name="io", bufs=2) as io_pool,
            tc.tile_pool(name="ps_s", bufs=3, space=bass.MemorySpace.PSUM) as ps_s,
            tc.tile_pool(name="ps_t", bufs=1, space=bass.MemorySpace.PSUM) as ps_t,
            tc.tile_pool(name="dram", bufs=2, space=bass.MemorySpace.DRAM) as dram_pool,
        ):
            # GPSIMD is used only for dma_gather (mlp library); load it once.
            nc.gpsimd.load_library(library_config.mlp)

            ident = persist.tile([128, 128], f32, tag="ident")
            nc.sync.dma_start(out=ident[:], in_=ident_d[:])

            rT = []
            for g in range(G):
                t = persist.tile([128, N], f32, tag=f"rT{g}")
                rT.append(t)

            def load_x():
                for g in range(G):
                    nc.sync.dma_start(out=rT[g][0:64, :], in_=x_d[g, :, :])
                    nc.sync.dma_start(out=rT[g][64:128, :], in_=x_d[g, :, :])

            load_x()

            def load_cbta(lg):
                cbta = cbta_pool.tile([128, K], f32, tag="cbta")
                nc.sync.dma_start(out=cbta[:], in_=cbta2_d[lg, :, :])
                nrm = nrm_pool.tile([128, K], f32, tag="nrm")
                src = bass.AP(
                    norms_d[:].tensor, lg * K, [[0, 128], [1, K]]
                )  # partition-stride-0 broadcast of the norms row
                nc.sync.dma_start(out=nrm[:], in_=src)
                return cbta, nrm

            amout = am_pool.tile([128, K], f32, tag="amout")

            def finish_group(pend):
                """Transpose gathered codewords and update the residual for a
                previous (l, g) step. Emitted one step late so the PE (strict
                in-order FIFO) never stalls waiting for that step's gather."""
                g, q, qT = pend
                qtp = ps_t.tile([64, NCHUNKS, 128], f32, tag="qtp")
                for j in range(NCHUNKS):
                    nc.tensor.transpose(qtp[:, j, :], q[:, j, :], ident[:])
                    nc.scalar.copy(
                        out=qT[:, j * 128 : (j + 1) * 128], in_=qtp[:, j, :]
                    )
                # residual update (DVE; gpsimd's tensor_tensor lives in a
                # different ucode library than dma_gather), then duplicate
                # the updated half into partitions 64-127 for row-group 2
                nc.vector.tensor_sub(rT[g][0:64, :], rT[g][0:64, :], qT[:])
                nc.sync.dma_start(out=rT[g][64:128, :], in_=rT[g][0:64, :])

            steps = [(l, g) for _ in range(repeats) for l in range(L) for g in range(G)]
            cbta_tiles = {0: load_cbta(steps[0][0] * G + steps[0][1])}
            pending = None
            for si, (l, g) in enumerate(steps):
                if si > 0 and l == 0 and g == 0:
                    # next benchmark repeat: reset residuals to x
                    if pending is not None:
                        finish_group(pending)
                        pending = None
                    load_x()
                lg = l * G + g
                cbta, nrm = cbta_tiles.pop(si)
                # prefetch the next step's codebook ahead of this step's
                # index-DMA chain (both go through the SP sequencer FIFO)
                if si + 1 < len(steps):
                    nl, ng = steps[si + 1]
                    cbta_tiles[si + 1] = load_cbta(nl * G + ng)

                Xf = small_pool.tile([128, NCHUNKS], f32, tag="Xf")
                Xu = small_pool.tile([128, NCHUNKS], u16, tag="Xu")
                q = q_pool.tile([128, NCHUNKS, D], f32, tag="q")
                qT = q_pool.tile([64, N], f32, tag="qT")

                for pj in range(NCHUNKS // 2):
                    j0, j1 = 2 * pj, 2 * pj + 1
                    sA = scores_pool.tile([128, K], f32, tag="scores")
                    sB = scores_pool.tile([128, K], f32, tag="scores")
                    for h in range(2):
                        psA = ps_s.tile([128, 2, 512], f32, tag="ps_s")
                        psB = ps_s.tile([128, 2, 512], f32, tag="ps_s")
                        for c in range(2):
                            kc = 2 * h + c
                            # row-packed fp32 pair: row-groups 0-63 / 64-127
                            # compute chunks j0 and j1 concurrently
                            nc.tensor.matmul(
                                psA[:, c, :],
                                rT[g][0:64, j0 * 128 : (j0 + 1) * 128],
                                cbta[0:64, kc * 512 : (kc + 1) * 512],
                                start=True,
                                stop=True,
                            )
                            nc.tensor.matmul(
                                psB[:, c, :],
                                rT[g][64:128, j1 * 128 : (j1 + 1) * 128],
                                cbta[64:128, kc * 512 : (kc + 1) * 512],
                                start=True,
                                stop=True,
                            )
                        nc.scalar.copy(
                            out=sA[:, h * 1024 : (h + 1) * 1024], in_=psA[:, :, :]
                        )
                        nc.scalar.copy(
                            out=sB[:, h * 1024 : (h + 1) * 1024], in_=psB[:, :, :]
                        )
                    # single-pass biased argmax (norms enter via Src1)
                    nc.vector._custom_dve(
                        argmax_op,
                        out=amout[:],
                        in0=sA[:],
                        in1=nrm[:],
                        accum_out=Xf[:, j0 : j0 + 1],
                    )
                    nc.vector._custom_dve(
                        argmax_op,
                        out=amout[:],
                        in0=sB[:],
                        in1=nrm[:],
                        accum_out=Xf[:, j1 : j1 + 1],
                    )
                # cast float indices to u16 for the gather index tile
                nc.scalar.copy(out=Xu[:], in_=Xf[:])

                # previous step's epilogue goes to the engine FIFOs here,
                # after this step's matmuls/argmaxes
                if pending is not None:
                    finish_group(pending)

                # Build the dma_gather index tile: list position i = pixel
                # n lives at T[n % 16, n // 16] (int16, replicated to all
                # 128 partitions). The 128-partition -> 16-partition fold
                # goes through a DRAM round trip with a strided re-read:
                # T[qq, j*8+a] = Xu[16a+qq, j].
                idxd = dram_pool.tile([128, NCHUNKS], u16, tag="idxd")
                nc.sync.dma_start(out=idxd[:], in_=Xu[:])
                T = small_pool.tile([128, 64], u16, tag="T")
                nc.sync.dma_start(
                    out=T[0:16, :].rearrange("q (j a) -> q j a", j=NCHUNKS),
                    in_=idxd[:].rearrange("(a q) j -> q j a", a=8),
                )
                nc.sync.dma_start(out=T[16:32, :], in_=T[0:16, :])
                nc.sync.dma_start(out=T[32:64, :], in_=T[0:32, :])
                nc.sync.dma_start(out=T[64:128, :], in_=T[0:64, :])

                # gather all 1024 nearest codewords for this (l, g)
                nc.gpsimd.dma_gather(
                    out_ap=q[:],
                    in_ap=cbrows_d[lg * K : (lg + 1) * K, :],
                    idxs_ap=T[:].bitcast(i16),
                    num_idxs=N,
                    num_idxs_reg=N,
                    elem_size=D,
                )
                pending = (g, q, qT)

            finish_group(pending)

            # out = x - r_final  (xhat accumulated implicitly)
            for g in range(G):
                xin = io_pool.tile([64, N], f32, tag="xin")
                nc.sync.dma_start(out=xin[:], in_=x_d[g, :, :])
                xout = io_pool.tile([64, N], f32, tag="xout")
                nc.vector.tensor_sub(xout[:], xin[:], rT[g][0:64, :])
                nc.sync.dma_start(out=out_d[g * 64 : (g + 1) * 64, :], in_=xout[:])

    nc.compile()
    return nc


def _prep_inputs(x, codebooks):
    x = np.ascontiguousarray(np.asarray(x, dtype=np.float32))
    cb = np.ascontiguousarray(np.asarray(codebooks, dtype=np.float32))
    cbta2 = np.empty((L * G, 128, K), dtype=np.float32)
    norms = np.empty((L * G, K), dtype=np.float32)
    for l in range(L):
        for g in range(G):
            ct = cb[l, g].T
            cbta2[l * G + g, 0:64, :] = ct
            cbta2[l * G + g, 64:128, :] = ct
            norms[l * G + g] = -0.5 * np.sum(
                cb[l, g].astype(np.float32) ** 2, axis=1, dtype=np.float32
            )
    cbrows = np.ascontiguousarray(cb.reshape(L * G * K, D))
    ident = np.eye(128, dtype=np.float32)
    in_maps = []
    for b in range(B):
        in_maps.append(
            {
                "x": np.ascontiguousarray(x[b].reshape(G, 64, N)),
                "cbta2": cbta2,
                "norms": norms,
                "cbrows": cbrows,
                "ident": ident,
            }
        )
    return in_maps


def kernel(x, codebooks, _trace=False, _trace_kwargs=None):
    from concourse.bass_utils import run_bass_kernel_spmd

    if "nc" not in _CACHE:
        _CACHE["nc"] = _build()
    nc = _CACHE["nc"]

    in_maps = _prep_inputs(x, codebooks)
    res = run_bass_kernel_spmd(
        nc,
        in_maps,
        core_ids=list(range(B)),
        trace=_trace,
        **(_trace_kwargs or {}),
    )
    _CACHE["last_result"] = res
    out = np.stack([np.asarray(res.results[b]["out"]) for b in range(B)], axis=0)
    return out.reshape(B, C, H, W).astype(np.float32)
